# revision 5
# baseline (speedup 1.0000x reference)
"""BoxConv2d Trainium2 kernel — split-fp8 DoubleRow edition.

Math (see fp16 baseline in kernel_fp16_baseline.py for the derivation):
    out[b, c*FN+f] = Wx[c,f] @ x[b,c] @ Wy[c,f]^T
with Wx/Wy the clamped-ramp band matrices built on the host from the box
parameters.

Speed trick: fp8e4 (E4M3) matmuls with MatmulPerfMode.DoubleRow contract
K=256 at 0.5 cycles/column — 4x the MAC rate of fp16.  Pure fp8 data loses
too much precision (~3e-2 rel), so the moving data is *residual-split*:
    x = a + b,     a = e4m3(x),  b = e4m3(x - a)
    V = W8x@a + W8x@b            (accumulated in the same fp32 PSUM)
    V = c + d,     c = e4m3(V),  d = e4m3(V - c)
    out = W8y-contraction over (c) + (d), again PSUM-accumulated.
Weights are single e4m3 (band entries are exactly 0/1 except <=2 boundary
fractions per row), giving ~5e-3 rel error overall at half the fp16 PE time
per stage pair (4x rate, 2x terms).

Stage 1:  psv[j, f, io]     = sum_p  x[p, j] * Wx[f][io, p]       (DR over pc)
Stage 2:  pso[jo, fi,b2,io] = sum_j  V8[j, f, io] * Wy[f][jo, j]  (DR over jh)
Stage 2 packs a batch PAIR (b2) into the moving free dim for N=512 matmuls.

Sharding: 4 channels per core, all 4 batches, box params folded into the
per-core weight images.
"""

import numpy as np

B, C, FN, H, W = 4, 32, 4, 256, 256
N_CORES = 8
C_PER_CORE = C // N_CORES  # 4

_PROGRAM_CACHE = {}


def _build_program():
    import concourse.bass as bass
    import concourse.tile as tile
    from concourse import bacc, mybir

    nc = bacc.Bacc("TRN2", target_bir_lowering=False, debug=False)
    f8 = mybir.dt.float8e4
    f16 = mybir.dt.float16
    f32 = mybir.dt.float32
    DR = mybir.MatmulPerfMode.DoubleRow

    # Host layouts (per core), all e4m3 unless noted:
    # xab[b, c][p, ab*512 + pc*256 + j] = split8(x[b, c, pc*128+p, j])
    # wx8[c][p, pc*1024 + fp*512 + fi*256 + io] = Wx[c, 2fp+fi][io, pc*128+p]
    # wy8[c][j, jc*1024 + f*256 + joh*128 + jo] = Wy[c, f][joh*128+jo, jc*128+j]
    # out[bp, c][jop, joh*2048 + fp*1024 + fi*512 + b2*256 + io] (fp16)
    #   = out_img[bp*2+b2, c*FN+2fp+fi, io, joh*128+jop]
    xab = nc.dram_tensor("xab", [B, C_PER_CORE, 128, 1024], f8,
                         kind="ExternalInput").ap()
    wx8 = nc.dram_tensor("wx8", [C_PER_CORE, 128, 2048], f8,
                         kind="ExternalInput").ap()
    wy8 = nc.dram_tensor("wy8", [C_PER_CORE, 128, 2048], f8,
                         kind="ExternalInput").ap()
    out = nc.dram_tensor("out", [2, C_PER_CORE, 128, 4096], f16,
                         kind="ExternalOutput").ap()

    with tile.TileContext(nc, pool_alloc_mode="queue") as tc:
        with (
            tc.tile_pool(name="wx", bufs=2) as wx_pool,
            tc.tile_pool(name="wy", bufs=2) as wy_pool,
            tc.tile_pool(name="xin", bufs=6) as x_pool,
            tc.tile_pool(name="cd", bufs=6) as cd_pool,
            tc.tile_pool(name="osb", bufs=3) as o_pool,
            tc.tile_pool(name="psv", bufs=2, space=bass.MemorySpace.PSUM) as psv_pool,
            tc.tile_pool(name="pso", bufs=2, space=bass.MemorySpace.PSUM) as pso_pool,
        ):
            # Warm the PE clock gate during initial DMA latency.
            warm_sb = x_pool.tile([128, 128], f16, tag="warm", name="warm")
            nc.vector.memset(warm_sb[:], 0.0)
            warm_ps = pso_pool.tile([128, 2, 2, 256], f32, tag="pso", name="pso")
            for _w in range(32):
                nc.tensor.matmul(warm_ps[:, 0, 0, :128], warm_sb[:], warm_sb[:],
                                 start=True, stop=True)

            ocast = [nc.scalar.copy, nc.vector.tensor_copy]
            ocast_i = 0

            xt0 = None
            for c in range(C_PER_CORE):
                if c == 0:
                    # first x tile on its own queue, 2-way split
                    xt0 = x_pool.tile([128, 2, 2, 256], f8, tag="x", name="x")
                    nc.gpsimd.dma_start(xt0[:, 0], xab[0, 0][:, :512])
                    nc.gpsimd.dma_start(xt0[:, 1], xab[0, 0][:, 512:])
                wx_t = wx_pool.tile([128, 2, 2, 512], f8, tag="wx", name="wx")
                nsp = 4 if c == 0 else 1
                stp = 2048 // nsp
                for q in range(nsp):
                    nc.sync.dma_start(
                        wx_t.rearrange("p a b n -> p (a b n)")[:, q * stp:(q + 1) * stp],
                        wx8[c][:, q * stp:(q + 1) * stp])
                wy_t = wy_pool.tile([128, 2, 4, 2, 128], f8, tag="wy", name="wy")
                nsp = 2 if c == 0 else 1
                stp = 2048 // nsp
                for q in range(nsp):
                    nc.sync.dma_start(
                        wy_t.rearrange("p a b c n -> p (a b c n)")[:, q * stp:(q + 1) * stp],
                        wy8[c][:, q * stp:(q + 1) * stp])

                c8t = {}
                d8t = {}
                for b in range(B):
                    bp, b2 = b // 2, b % 2
                    if b2 == 0:
                        c8t[bp] = cd_pool.tile([128, 2, 4, 2, 256], f8,
                                               tag="c8", name="c8")
                        d8t[bp] = cd_pool.tile([128, 2, 4, 2, 256], f8,
                                               tag="d8", name="d8")
                    if c == 0 and b == 0:
                        xt = xt0
                    else:
                        xt = x_pool.tile([128, 2, 2, 256], f8, tag="x", name="x")
                        nc.gpsimd.dma_start(xt.rearrange("p a b n -> p (a b n)")[:],
                                            xab[b, c])

                    # ---- stage 1: psv[j, f, io] over both pc chunks (DR) ----
                    for jh in range(2):
                        psv = psv_pool.tile([128, 4, 256], f32, tag="psv",
                                            name="psv")
                        lhs_a = xt[:, 0, :, jh * 128:(jh + 1) * 128]
                        lhs_b = xt[:, 1, :, jh * 128:(jh + 1) * 128]
                        for fp in range(2):
                            nc.tensor.matmul(psv[:, 2 * fp:2 * fp + 2, :],
                                             lhs_a, wx_t[:, :, fp, :],
                                             start=True, stop=False,
                                             perf_mode=DR)
                        for fp in range(2):
                            nc.tensor.matmul(psv[:, 2 * fp:2 * fp + 2, :],
                                             lhs_b, wx_t[:, :, fp, :],
                                             start=False, stop=True,
                                             perf_mode=DR)
                        # split V = c + d (c on ACT, d on DVE)
                        cdst = c8t[bp][:, jh, :, b2, :]
                        nc.scalar.copy(cdst, psv[:, :, :])
                        nc.vector.tensor_sub(d8t[bp][:, jh, :, b2, :],
                                             psv[:, :, :], cdst)

                    # ---- stage 2 for this batch pair, once both b2 done ----
                    if b2 == 1:
                        osb = o_pool.tile([128, 2, 2, 2, 2, 256], f16,
                                          tag="o", name="osb")
                        for fp in range(2):
                            for joh in range(2):
                                pso = pso_pool.tile([128, 2, 2, 256], f32,
                                                    tag="pso", name="pso")
                                for fi in range(2):
                                    f = 2 * fp + fi
                                    wyl = wy_t[:, :, f, joh, :]
                                    nc.tensor.matmul(
                                        pso[:, fi, :, :], wyl,
                                        c8t[bp][:, :, f, :, :],
                                        start=True, stop=False, perf_mode=DR)
                                    nc.tensor.matmul(
                                        pso[:, fi, :, :], wyl,
                                        d8t[bp][:, :, f, :, :],
                                        start=False, stop=True, perf_mode=DR)
                                eng = ocast[ocast_i % 2]
                                ocast_i += 1
                                eng(osb[:, joh, fp], pso[:])
                        last = c == C_PER_CORE - 1 and bp == 1
                        nsp = 2 if last else 1
                        stp = 4096 // nsp
                        for q in range(nsp):
                            nc.sync.dma_start(
                                out[bp, c][:, q * stp:(q + 1) * stp],
                                osb.rearrange("p a b c d n -> p (a b c d n)")[:, q * stp:(q + 1) * stp])

    nc.compile()
    return nc


def _get_program():
    if "nc" not in _PROGRAM_CACHE:
        _PROGRAM_CACHE["nc"] = _build_program()
    return _PROGRAM_CACHE["nc"]


def _band(mn, mx, dim):
    i = np.arange(dim, dtype=np.float64)[:, None]
    p = np.arange(dim, dtype=np.float64)[None, :]
    lo = i + float(mn)
    hi = i + float(mx) + 1.0
    return np.clip(p + 1.0 - lo, 0.0, 1.0) - np.clip(p + 1.0 - hi, 0.0, 1.0)


def _prepare_in_maps(input, x_min, x_max, y_min, y_max):
    import ml_dtypes
    E4 = ml_dtypes.float8_e4m3

    x32 = input.astype(np.float32)
    a8 = x32.astype(E4)
    b8 = (x32 - a8.astype(np.float32)).astype(E4)
    # [B, C, pc, p, j] -> [B, C, p, (ab, pc, j)]
    def pack_x(arr):
        return (arr.reshape(B, C, 2, 128, 256).transpose(0, 1, 3, 2, 4)
                .reshape(B, C, 128, 512))
    ap, bps = pack_x(a8), pack_x(b8)
    xab_full = np.concatenate([ap, bps], axis=3)  # [B, C, 128, 1024]

    in_maps = []
    for core in range(N_CORES):
        c0 = core * C_PER_CORE
        wx8 = np.empty((C_PER_CORE, 128, 2048), dtype=E4)
        wy8 = np.empty((C_PER_CORE, 128, 2048), dtype=E4)
        for cl in range(C_PER_CORE):
            ch = c0 + cl
            for f in range(FN):
                Wx = _band(x_min[ch, f], x_max[ch, f], H).astype(E4)
                Wy = _band(y_min[ch, f], y_max[ch, f], W).astype(E4)
                fp, fi = f // 2, f % 2
                # wx8[p, pc*1024 + fp*512 + fi*256 + io] = Wx[io, pc*128+p]
                for pc in range(2):
                    base = pc * 1024 + fp * 512 + fi * 256
                    wx8[cl, :, base:base + 256] = Wx[:, pc * 128:(pc + 1) * 128].T
                # wy8[j, jc*1024 + f*256 + joh*128 + jo] = Wy[joh*128+jo, jc*128+j]
                for jc in range(2):
                    for joh in range(2):
                        base = jc * 1024 + f * 256 + joh * 128
                        wy8[cl, :, base:base + 128] = \
                            Wy[joh * 128:(joh + 1) * 128,
                               jc * 128:(jc + 1) * 128].T
        in_maps.append({
            "xab": np.ascontiguousarray(xab_full[:, c0:c0 + C_PER_CORE]),
            "wx8": wx8,
            "wy8": wy8,
        })
    return in_maps


def run(input, x_min, x_max, y_min, y_max, trace=False):
    from concourse.bass_utils import run_bass_kernel_spmd

    nc = _get_program()
    in_maps = _prepare_in_maps(
        np.asarray(input, dtype=np.float32),
        np.asarray(x_min, dtype=np.float64),
        np.asarray(x_max, dtype=np.float64),
        np.asarray(y_min, dtype=np.float64),
        np.asarray(y_max, dtype=np.float64),
    )
    res = run_bass_kernel_spmd(nc, in_maps, list(range(N_CORES)), trace=trace)
    parts = []
    for i in range(N_CORES):
        o = res.results[i]["out"].astype(np.float32).reshape(
            2, C_PER_CORE, 128, 2, 2, 2, 2, 256)
        # dims: bp, cl, jop, joh, fp, fi, b2, io
        o = o.transpose(0, 6, 1, 4, 5, 7, 3, 2).reshape(
            B, C_PER_CORE * FN, 256, 256)
        parts.append(o)
    full = np.ascontiguousarray(np.concatenate(parts, axis=1))
    return full, res


def kernel(input, x_min, x_max, y_min, y_max):
    full, _ = run(input, x_min, x_max, y_min, y_max)
    return full


# revision 6
# speedup vs baseline: 1.1063x; 1.1063x over previous
"""BoxConv2d Trainium2 kernel.

Reference computes, per (c, f) box and batch b:
    out[b, c*FN+f, i, j] = integral of x[b, c] over the continuous window
        rows [i + x_min, i + x_max + 1) x cols [j + y_min, j + y_max + 1),
    with window coordinates clipped to [0, H] x [0, W] (bilinear sampling of
    the integral image is exact for piecewise-constant images).

That is exactly a separable band matmul with clamped-ramp overlap weights:
    Wx[i, p] = clamp01(p + 1 - (i + x_min)) - clamp01(p + 1 - (i + x_max + 1))
    Wy[j, q] = clamp01(q + 1 - (j + y_min)) - clamp01(q + 1 - (j + y_max + 1))
    out[b, cf] = Wx @ x[b, c] @ Wy^T

The Wx/Wy matrices depend only on the tiny box parameters, so they are built
on the host and shipped to the device; the device kernel is pure TensorE
matmuls in fp16 (fp32 PSUM accumulation), which numpy-validates to ~3e-4
relative error against the fp32 reference.

Sharding: channels across the 8 cores (4 channels/core, all 4 batches), box
parameters replicated per-core as part of each core's W shard.

Step 1 (x side):  V^B[j, f*256+io] = sum_p x[p, j] * Wx[f][io, p]
    lhsT (stationary) = x chunk [p-chunk, j-half], rhs = WxT [p-chunk, 2f*io].
Step 2 (y side):  out[ih*128+io, jo] = sum_j V[j, ...] * Wy[f][jo, j]
    lhsT = V chunk [j-chunk, io-half], rhs = WyT [j-chunk, jo].
"""

import numpy as np

B, C, FN, H, W = 4, 32, 4, 256, 256
N_CORES = 8
C_PER_CORE = C // N_CORES  # 4 channels per core

_PROGRAM_CACHE = {}


def _build_program():
    """Build (once) the SPMD Bass program run identically on all 8 cores."""
    import concourse.bass as bass
    import concourse.tile as tile
    from concourse import bacc, mybir

    nc = bacc.Bacc("TRN2", target_bir_lowering=False, debug=False)
    f16 = mybir.dt.float16
    f32 = mybir.dt.float32

    # Per-core inputs, host-laid-out so every DMA is one contiguous 2D copy:
    # x16[b, c, p, pc*256 + j]          = x[b, c, pc*128 + p, j]
    # wxt[c, p, (fp*2+pc)*512 + fi*256 + io] = Wx[c, 2fp+fi][io, pc*128 + p]
    # wyt[c, j, (f*2+jc)*256 + jo]      = Wy[c, f][jo, jc*128 + j]
    x16 = nc.dram_tensor("x16", [B, C_PER_CORE, 128, 512], f16,
                         kind="ExternalInput").ap()
    wxt = nc.dram_tensor("wxt", [C_PER_CORE, 128, 2048], f16,
                         kind="ExternalInput").ap()
    wyt = nc.dram_tensor("wyt", [C_PER_CORE, 128, 2048], f16,
                         kind="ExternalInput").ap()
    # out_dev[b, c, p, f*512 + a*256 + jo] = out[b, c*FN+f, a*128+p, jo]
    # (host transposes back; keeps store DMAs fully contiguous per partition)
    # fp16 output (|out| <~1e3, fp16 quantization ~5e-4 rel; host upcasts):
    # halves store traffic, and the kernel tail is store-drain bound.
    out = nc.dram_tensor("out", [B, C_PER_CORE, 128, 2048], f16,
                         kind="ExternalOutput").ap()

    with tile.TileContext(nc, pool_alloc_mode="queue") as tc:
        with (
            tc.tile_pool(name="wx", bufs=3) as wx_pool,
            tc.tile_pool(name="wy", bufs=3) as wy_pool,
            tc.tile_pool(name="xin", bufs=10) as x_pool,
            tc.tile_pool(name="v", bufs=8) as v_pool,
            tc.tile_pool(name="osb", bufs=6) as o_pool,
            tc.tile_pool(name="psv", bufs=2, space=bass.MemorySpace.PSUM) as psv_pool,
            tc.tile_pool(name="pso", bufs=4, space=bass.MemorySpace.PSUM) as pso_pool,
        ):
            # Warm the PE clock gate (HAM) during the initial load
            # latency with dependency-free matmuls on scratch data.
            warm_sb = x_pool.tile([128, 128], f16, tag="warm_sb", name="warm_sb")
            nc.vector.memset(warm_sb[:], 0.0)
            warm_ps = pso_pool.tile([128, 512], f32, tag="pso", name="pso")
            for _w in range(32):
                nc.tensor.matmul(warm_ps[:, :128], warm_sb[:], warm_sb[:],
                                 start=True, stop=True)

            xt0 = None
            for c in range(C_PER_CORE):
                # First x tile + first Wx chunk are on the critical path:
                # issue on separate engines/queues, Wx 4-way-split in MM use
                # order so the first matmul (subtile deps) waits only on the
                # first 128KB.  (Per-queue DMA BW is ~110GB/s.)
                if c == 0:
                    xt0 = x_pool.tile([128, 512], f16, tag="x", name="x")
                    nc.gpsimd.dma_start(xt0[:, :256], x16[0, 0][:, :256])
                    nc.gpsimd.dma_start(xt0[:, 256:], x16[0, 0][:, 256:])
                wx_t = wx_pool.tile([128, 2048], f16, tag="wx", name="wx")
                nsplit = 4 if c == 0 else 1
                step = 2048 // nsplit
                for q in range(nsplit):
                    nc.gpsimd.dma_start(wx_t[:, q * step:(q + 1) * step],
                                        wxt[c][:, q * step:(q + 1) * step])
                wy_t = wy_pool.tile([128, 2048], f16, tag="wy", name="wy")
                nsplit = 2 if c == 0 else 1
                step = 2048 // nsplit
                for q in range(nsplit):
                    nc.gpsimd.dma_start(wy_t[:, q * step:(q + 1) * step],
                                        wyt[c][:, q * step:(q + 1) * step])

                for b in range(B):
                    if c == 0 and b == 0:
                        xt = xt0
                    else:
                        xt = x_pool.tile([128, 512], f16, tag="x", name="x")
                        nc.gpsimd.dma_start(xt[:], x16[b, c])

                    # Step 1: psv holds both f-pairs (2 PSUM banks); one
                    # big PSUM->SBUF cast per jh, alternating engine.
                    vt = [v_pool.tile([128, 1024], f16, tag="v", name="v")
                          for _jh in range(2)]
                    for jh in range(2):
                        psv = psv_pool.tile([128, 1024], f32, tag="psv",
                                            name="psv")
                        for fp in range(2):
                            for pc in range(2):
                                nc.tensor.matmul(
                                    psv[:, fp * 512:(fp + 1) * 512],
                                    xt[:, pc * 256 + jh * 128:
                                       pc * 256 + jh * 128 + 128],
                                    wx_t[:, (fp * 2 + pc) * 512:
                                         (fp * 2 + pc) * 512 + 512],
                                    start=(pc == 0),
                                    stop=(pc == 1),
                                )
                        eng = nc.vector.tensor_copy if jh == 0 else nc.scalar.copy
                        eng(vt[jh][:], psv[:])

                    # Step 2
                    osb = o_pool.tile([128, 2048], f16, tag="o", name="osb")
                    for f in range(FN):
                        pso = pso_pool.tile([128, 512], f32, tag="pso",
                                            name="pso")
                        for ih in range(2):
                            for jc in range(2):
                                nc.tensor.matmul(
                                    pso[:, ih * 256:(ih + 1) * 256],
                                    vt[jc][:, f * 256 + ih * 128:
                                           f * 256 + ih * 128 + 128],
                                    wy_t[:, (f * 2 + jc) * 256:
                                         (f * 2 + jc) * 256 + 256],
                                    start=(jc == 0),
                                    stop=(jc == 1),
                                )
                        dst = osb[:, f * 512:(f + 1) * 512]
                        eng = nc.vector.tensor_copy if f % 2 == 0 else nc.scalar.copy
                        eng(dst[:], pso[:])
                        if c == C_PER_CORE - 1 and b == B - 1:
                            # final tile: store each f-chunk as soon as it is
                            # copied, shortening the kernel tail
                            nc.sync.dma_start(
                                out[b, c][:, f * 512:(f + 1) * 512], dst[:])
                    if not (c == C_PER_CORE - 1 and b == B - 1):
                        # contiguous store on sync, split across DMA queues
                        # (4-way near the end to shorten the final drain)
                        nsp = 4 if (c == C_PER_CORE - 1 and b == B - 2) else 2
                        stp = 2048 // nsp
                        for q in range(nsp):
                            nc.sync.dma_start(
                                out[b, c][:, q * stp:(q + 1) * stp],
                                osb[:, q * stp:(q + 1) * stp])

    nc.compile()
    return nc


def _get_program():
    if "nc" not in _PROGRAM_CACHE:
        _PROGRAM_CACHE["nc"] = _build_program()
    return _PROGRAM_CACHE["nc"]


def _band(mn, mx, dim):
    """Overlap weights W[i, p] of clipped window [i+mn, i+mx+1) with cell
    [p, p+1), built in fp64."""
    i = np.arange(dim, dtype=np.float64)[:, None]
    p = np.arange(dim, dtype=np.float64)[None, :]
    lo = i + float(mn)
    hi = i + float(mx) + 1.0
    return np.clip(p + 1.0 - lo, 0.0, 1.0) - np.clip(p + 1.0 - hi, 0.0, 1.0)


def _prepare_in_maps(input, x_min, x_max, y_min, y_max):
    # x16[b, c, p, pc*256 + j] = x[b, c, pc*128 + p, j]
    x16_full = np.ascontiguousarray(
        input.astype(np.float16).reshape(B, C, 2, 128, 256)
        .transpose(0, 1, 3, 2, 4).reshape(B, C, 128, 512))

    in_maps = []
    for core in range(N_CORES):
        c0 = core * C_PER_CORE
        wxt = np.empty((C_PER_CORE, 128, 2048), dtype=np.float16)
        wyt = np.empty((C_PER_CORE, 128, 2048), dtype=np.float16)
        for cl in range(C_PER_CORE):
            c = c0 + cl
            for f in range(FN):
                WxT = _band(x_min[c, f], x_max[c, f], H).T.astype(np.float16)
                WyT = _band(y_min[c, f], y_max[c, f], W).T.astype(np.float16)
                fp, fi = f // 2, f % 2
                for pc in range(2):
                    base = (fp * 2 + pc) * 512 + fi * 256
                    wxt[cl, :, base:base + 256] = WxT[pc * 128:(pc + 1) * 128]
                for jc in range(2):
                    base = (f * 2 + jc) * 256
                    wyt[cl, :, base:base + 256] = WyT[jc * 128:(jc + 1) * 128]
        in_maps.append({
            "x16": np.ascontiguousarray(x16_full[:, c0:c0 + C_PER_CORE]),
            "wxt": wxt,
            "wyt": wyt,
        })
    return in_maps


def run(input, x_min, x_max, y_min, y_max, trace=False):
    """Run the SPMD kernel; returns (full_output, BassKernelResults)."""
    from concourse.bass_utils import run_bass_kernel_spmd

    nc = _get_program()
    in_maps = _prepare_in_maps(
        np.asarray(input, dtype=np.float32),
        np.asarray(x_min, dtype=np.float64),
        np.asarray(x_max, dtype=np.float64),
        np.asarray(y_min, dtype=np.float64),
        np.asarray(y_max, dtype=np.float64),
    )
    res = run_bass_kernel_spmd(nc, in_maps, list(range(N_CORES)), trace=trace)
    # out_dev[b, c, p, f*512 + a*256 + jo] -> out[b, c*FN+f, a*128+p, jo]
    parts = []
    for i in range(N_CORES):
        o = res.results[i]["out"].astype(np.float32).reshape(
            B, C_PER_CORE, 128, FN, 2, 256)
        parts.append(o.transpose(0, 1, 3, 4, 2, 5).reshape(
            B, C_PER_CORE * FN, 256, 256))
    full = np.ascontiguousarray(np.concatenate(parts, axis=1))
    return full, res


def kernel(input, x_min, x_max, y_min, y_max):
    full, _ = run(input, x_min, x_max, y_min, y_max)
    return full



# revision 9
# speedup vs baseline: 1.1209x; 1.0132x over previous
"""BoxConv2d Trainium2 kernel.

Reference computes, per (c, f) box and batch b:
    out[b, c*FN+f, i, j] = integral of x[b, c] over the continuous window
        rows [i + x_min, i + x_max + 1) x cols [j + y_min, j + y_max + 1),
    with window coordinates clipped to [0, H] x [0, W] (bilinear sampling of
    the integral image is exact for piecewise-constant images).

That is exactly a separable band matmul with clamped-ramp overlap weights:
    Wx[i, p] = clamp01(p + 1 - (i + x_min)) - clamp01(p + 1 - (i + x_max + 1))
    Wy[j, q] = clamp01(q + 1 - (j + y_min)) - clamp01(q + 1 - (j + y_max + 1))
    out[b, cf] = Wx @ x[b, c] @ Wy^T

The Wx/Wy matrices depend only on the tiny box parameters, so they are built
on the host and shipped to the device; the device kernel is pure TensorE
matmuls in fp16 (fp32 PSUM accumulation), which numpy-validates to ~6e-4
relative error against the fp32 reference.

Sharding: channels across the 8 cores (4 channels/core, all 4 batches), box
parameters replicated per-core as part of each core's W shard.

Step 1 (x side):  V^B[j, f*256+io] = sum_p x[p, j] * Wx[f][io, p]
    lhsT (stationary) = x chunk [p-chunk, j-half], rhs = WxT [p-chunk, 2f*io].
Step 2 (y side):  out[ih*128+io, jo] = sum_j V[j, ...] * Wy[f][jo, j]
    lhsT = V chunk [j-chunk, io-half], rhs = WyT [j-chunk, jo].

DMA ring assignment (each issuing engine owns a hw queue; per-ring BW is
limited, stores alone are 8MB/core): x tiles -> gpsimd, WxT -> tensor,
WyT -> scalar, output stores alternate sync/gpsimd; the last tiles fan the
final stores across four queues to shorten the drain.
"""

import numpy as np

B, C, FN, H, W = 4, 32, 4, 256, 256
N_CORES = 8
C_PER_CORE = C // N_CORES  # 4 channels per core

_PROGRAM_CACHE = {}


def _build_program():
    """Build (once) the SPMD Bass program run identically on all 8 cores."""
    import concourse.bass as bass
    import concourse.tile as tile
    from concourse import bacc, mybir

    nc = bacc.Bacc("TRN2", target_bir_lowering=False, debug=False)
    f16 = mybir.dt.float16
    f32 = mybir.dt.float32

    # Per-core inputs, host-laid-out so every DMA is one contiguous 2D copy:
    # x16[b, c, p, pc*256 + j]          = x[b, c, pc*128 + p, j]
    # wxt[c, p, (fp*2+pc)*512 + fi*256 + io] = Wx[c, 2fp+fi][io, pc*128 + p]
    # wyt[c, j, (f*2+jc)*256 + jo]      = Wy[c, f][jo, jc*128 + j]
    x16 = nc.dram_tensor("x16", [B, C_PER_CORE, 128, 512], f16,
                         kind="ExternalInput").ap()
    wxt = nc.dram_tensor("wxt", [C_PER_CORE, 128, 2048], f16,
                         kind="ExternalInput").ap()
    wyt = nc.dram_tensor("wyt", [C_PER_CORE, 128, 2048], f16,
                         kind="ExternalInput").ap()
    # out_dev[b, c, p, f*512 + a*256 + jo] = out[b, c*FN+f, a*128+p, jo]
    # fp16 output (host upcasts); halves store traffic.
    out = nc.dram_tensor("out", [B, C_PER_CORE, 128, 2048], f16,
                         kind="ExternalOutput").ap()

    with tile.TileContext(nc, pool_alloc_mode="queue") as tc:
        with (
            tc.tile_pool(name="wx", bufs=3) as wx_pool,
            tc.tile_pool(name="wy", bufs=3) as wy_pool,
            tc.tile_pool(name="xin", bufs=10) as x_pool,
            tc.tile_pool(name="v", bufs=8) as v_pool,
            tc.tile_pool(name="osb", bufs=6) as o_pool,
            tc.tile_pool(name="psv", bufs=2, space=bass.MemorySpace.PSUM) as psv_pool,
            tc.tile_pool(name="pso", bufs=4, space=bass.MemorySpace.PSUM) as pso_pool,
        ):
            # Warm the PE clock gate (HAM) during the initial load
            # latency with dependency-free matmuls on scratch data.
            warm_sb = x_pool.tile([128, 128], f16, tag="warm_sb", name="warm_sb")
            nc.vector.memset(warm_sb[:], 0.0)
            warm_ps = pso_pool.tile([128, 512], f32, tag="pso", name="pso")
            for _w in range(24):
                nc.tensor.matmul(warm_ps[:, :128], warm_sb[:], warm_sb[:],
                                 start=True, stop=True)

            store_engs = [nc.sync, nc.gpsimd]
            xt0 = None
            for c in range(C_PER_CORE):
                # First x tile + first Wx chunks are on the critical path:
                # spread them across queues so the first real matmul only
                # waits on two parallel 128KB transfers.
                if c == 0:
                    xt0 = x_pool.tile([128, 512], f16, tag="x", name="x")
                    nc.gpsimd.dma_start(xt0[:, :256], x16[0, 0][:, :256])
                    nc.gpsimd.dma_start(xt0[:, 256:], x16[0, 0][:, 256:])
                wx_t = wx_pool.tile([128, 2048], f16, tag="wx", name="wx")
                if c == 0:
                    # fp0 needs cols [0:1024) (pc0+pc1); q0 lands first on
                    # its own queue so MM(jh0,fp0,pc0) starts earliest.
                    wx_engs = [nc.scalar, nc.sync, nc.scalar, nc.sync]
                    for q in range(4):
                        wx_engs[q].dma_start(wx_t[:, q * 512:(q + 1) * 512],
                                             wxt[c][:, q * 512:(q + 1) * 512])
                else:
                    nc.scalar.dma_start(wx_t[:], wxt[c])
                wy_t = wy_pool.tile([128, 2048], f16, tag="wy", name="wy")
                nsplit = 2 if c == 0 else 1
                step = 2048 // nsplit
                for q in range(nsplit):
                    nc.scalar.dma_start(wy_t[:, q * step:(q + 1) * step],
                                        wyt[c][:, q * step:(q + 1) * step])

                for b in range(B):
                    if c == 0 and b == 0:
                        xt = xt0
                    else:
                        xt = x_pool.tile([128, 512], f16, tag="x", name="x")
                        nc.gpsimd.dma_start(xt[:], x16[b, c])

                    # Step 1: psv holds both f-pairs (2 PSUM banks); one
                    # big PSUM->SBUF cast per jh, alternating engine.
                    vt = [v_pool.tile([128, 1024], f16, tag="v", name="v")
                          for _jh in range(2)]
                    for jh in range(2):
                        psv = psv_pool.tile([128, 1024], f32, tag="psv",
                                            name="psv")
                        for fp in range(2):
                            for pc in range(2):
                                nc.tensor.matmul(
                                    psv[:, fp * 512:(fp + 1) * 512],
                                    xt[:, pc * 256 + jh * 128:
                                       pc * 256 + jh * 128 + 128],
                                    wx_t[:, (fp * 2 + pc) * 512:
                                         (fp * 2 + pc) * 512 + 512],
                                    start=(pc == 0),
                                    stop=(pc == 1),
                                )
                        eng = nc.vector.tensor_copy if jh == 0 else nc.scalar.copy
                        eng(vt[jh][:], psv[:])

                    # Step 2
                    osb = o_pool.tile([128, 2048], f16, tag="o", name="osb")
                    last = c == C_PER_CORE - 1 and b == B - 1
                    for f in range(FN):
                        pso = pso_pool.tile([128, 512], f32, tag="pso",
                                            name="pso")
                        for ih in range(2):
                            for jc in range(2):
                                nc.tensor.matmul(
                                    pso[:, ih * 256:(ih + 1) * 256],
                                    vt[jc][:, f * 256 + ih * 128:
                                           f * 256 + ih * 128 + 128],
                                    wy_t[:, (f * 2 + jc) * 256:
                                         (f * 2 + jc) * 256 + 256],
                                    start=(jc == 0),
                                    stop=(jc == 1),
                                )
                        dst = osb[:, f * 512:(f + 1) * 512]
                        eng = nc.vector.tensor_copy if f % 2 == 0 else nc.scalar.copy
                        eng(dst[:], pso[:])
                        if last:
                            # final tile: store each f-chunk as soon as it is
                            # copied, one store per queue, shortening the tail
                            feng = [nc.sync, nc.gpsimd, nc.scalar, nc.sync][f]
                            feng.dma_start(
                                out[b, c][:, f * 512:(f + 1) * 512], dst[:])
                    if not last:
                        # contiguous store, alternating sync/gpsimd rings
                        # (4-way near the end to shorten the final drain)
                        if c == C_PER_CORE - 1 and b == B - 2:
                            for q in range(4):
                                store_engs[q % 2].dma_start(
                                    out[b, c][:, q * 512:(q + 1) * 512],
                                    osb[:, q * 512:(q + 1) * 512])
                        else:
                            eng = store_engs[(c * B + b) % 2]
                            for q in range(2):
                                eng.dma_start(
                                    out[b, c][:, q * 1024:(q + 1) * 1024],
                                    osb[:, q * 1024:(q + 1) * 1024])

    nc.compile()
    return nc


def _get_program():
    if "nc" not in _PROGRAM_CACHE:
        _PROGRAM_CACHE["nc"] = _build_program()
    return _PROGRAM_CACHE["nc"]


def _band(mn, mx, dim):
    """Overlap weights W[i, p] of clipped window [i+mn, i+mx+1) with cell
    [p, p+1), built in fp64."""
    i = np.arange(dim, dtype=np.float64)[:, None]
    p = np.arange(dim, dtype=np.float64)[None, :]
    lo = i + float(mn)
    hi = i + float(mx) + 1.0
    return np.clip(p + 1.0 - lo, 0.0, 1.0) - np.clip(p + 1.0 - hi, 0.0, 1.0)


def _prepare_in_maps(input, x_min, x_max, y_min, y_max):
    # x16[b, c, p, pc*256 + j] = x[b, c, pc*128 + p, j]
    x16_full = np.ascontiguousarray(
        input.astype(np.float16).reshape(B, C, 2, 128, 256)
        .transpose(0, 1, 3, 2, 4).reshape(B, C, 128, 512))

    in_maps = []
    for core in range(N_CORES):
        c0 = core * C_PER_CORE
        wxt = np.empty((C_PER_CORE, 128, 2048), dtype=np.float16)
        wyt = np.empty((C_PER_CORE, 128, 2048), dtype=np.float16)
        for cl in range(C_PER_CORE):
            c = c0 + cl
            for f in range(FN):
                WxT = _band(x_min[c, f], x_max[c, f], H).T.astype(np.float16)
                WyT = _band(y_min[c, f], y_max[c, f], W).T.astype(np.float16)
                fp, fi = f // 2, f % 2
                for pc in range(2):
                    base = (fp * 2 + pc) * 512 + fi * 256
                    wxt[cl, :, base:base + 256] = WxT[pc * 128:(pc + 1) * 128]
                for jc in range(2):
                    base = (f * 2 + jc) * 256
                    wyt[cl, :, base:base + 256] = WyT[jc * 128:(jc + 1) * 128]
        in_maps.append({
            "x16": np.ascontiguousarray(x16_full[:, c0:c0 + C_PER_CORE]),
            "wxt": wxt,
            "wyt": wyt,
        })
    return in_maps


def run(input, x_min, x_max, y_min, y_max, trace=False):
    """Run the SPMD kernel; returns (full_output, BassKernelResults)."""
    from concourse.bass_utils import run_bass_kernel_spmd

    nc = _get_program()
    in_maps = _prepare_in_maps(
        np.asarray(input, dtype=np.float32),
        np.asarray(x_min, dtype=np.float64),
        np.asarray(x_max, dtype=np.float64),
        np.asarray(y_min, dtype=np.float64),
        np.asarray(y_max, dtype=np.float64),
    )
    res = run_bass_kernel_spmd(nc, in_maps, list(range(N_CORES)), trace=trace)
    # out_dev[b, c, p, f*512 + a*256 + jo] -> out[b, c*FN+f, a*128+p, jo]
    parts = []
    for i in range(N_CORES):
        o = res.results[i]["out"].astype(np.float32).reshape(
            B, C_PER_CORE, 128, FN, 2, 256)
        parts.append(o.transpose(0, 1, 3, 4, 2, 5).reshape(
            B, C_PER_CORE * FN, 256, 256))
    full = np.ascontiguousarray(np.concatenate(parts, axis=1))
    return full, res


def kernel(input, x_min, x_max, y_min, y_max):
    full, _ = run(input, x_min, x_max, y_min, y_max)
    return full
